# revision 6
# baseline (speedup 1.0000x reference)
"""GNN message-passing kernel for Trainium2 (8 NeuronCores, SPMD).

Computes: h = x @ W + b;  out[r] = sum_{e: rows[e]==r} vals[e] * h[cols[e]]

Strategy (dest-sharded):
  - Destinations (output rows) are sharded across 8 cores; each core owns a
    contiguous range of D = blocks_per_core*128 rows.
  - Linearity: out = segsum(vals * x[cols]) @ W + rowsum ⊗ b, so the dense
    transform runs per-core on owned rows only; no h materialization and no
    cross-core exchange (each core's dma_gather reads the replicated fp16 x
    straight from its HBM copy).
  - Edges are host-sorted by (dest superblock, source chunk, dest block) and
    padded into 128-edge tiles on a tile schedule shared by all 8 cores
    (SPMD: one program).  dma_gather (int16 indices -> 4 source chunks of
    <=32768 rows) pulls the 512B fp16 source rows for a whole
    (superblock, chunk) group in one shot.
  - Scatter into destinations uses a per-tile one-hot matmul: P[e,d] =
    vals[e] * (dest_local[e] == d), built with a single dual-op
    tensor_scalar against an iota constant; PSUM accumulates aggT[f,d] per
    dest block (lhsT = gathered features, rhs = P).
  - Epilogue per block: aggT -> SBUF fp16, out = aggT.T @ W (+ rank-1
    rowsum ⊗ bias matmul), copy f32 out rows to HBM.
"""

import sys

for _p in ("/opt/trn_rl_repo", "/root/.axon_site/_ro/trn_rl_repo"):
    if _p not in sys.path:
        sys.path.append(_p)

import numpy as np
from contextlib import ExitStack
from dataclasses import dataclass

import concourse.bass as bass
import concourse.tile as tile
from concourse import bacc, mybir
from concourse import library_config
from concourse.bass_utils import run_bass_kernel_spmd

F16 = mybir.dt.float16
F32 = mybir.dt.float32
I16 = mybir.dt.int16


@dataclass
class Cfg:
    n_nodes: int = 100000
    feat: int = 256            # in dim == out dim
    n_cores: int = 8
    blk: int = 128             # dest block (PSUM partitions)
    sb_blocks: int = 4         # dest blocks per superblock
    chunk_size: int = 25088    # source chunk (< 32768 for int16 idx)
    gath_bufs: int = 6

    @property
    def blocks_per_core(self) -> int:
        total = -(-self.n_nodes // self.blk)          # ceil
        total = -(-total // self.n_cores) * self.n_cores
        return total // self.n_cores

    @property
    def dests_per_core(self) -> int:
        return self.blocks_per_core * self.blk

    @property
    def n_sb(self) -> int:
        return -(-self.blocks_per_core // self.sb_blocks)

    @property
    def n_chunks(self) -> int:
        return -(-self.n_nodes // self.chunk_size)


def _prepare(rows, cols, vals, cfg: Cfg):
    """Sort/pad edges into the shared tile schedule; build per-core arrays."""
    blk, D = cfg.blk, cfg.dests_per_core
    NSB, NC, SBB = cfg.n_sb, cfg.n_chunks, cfg.sb_blocks
    n_cores = cfg.n_cores

    rows = np.asarray(rows, dtype=np.int64)
    cols = np.asarray(cols, dtype=np.int64)
    vals = np.asarray(vals, dtype=np.float32)

    core = rows // D
    block_local = (rows % D) // blk
    sb = block_local // SBB
    b_in = block_local % SBB
    chunk = cols // cfg.chunk_size

    gpc = NSB * NC * SBB                      # groups per core
    g = ((core * NSB + sb) * NC + chunk) * SBB + b_in
    counts = np.bincount(g, minlength=n_cores * gpc)
    tiles = -(-counts.reshape(n_cores, NSB, NC, SBB) // blk)
    tiles = tiles.max(axis=0)                 # [NSB, NC, SBB] shared schedule

    tile_off = np.zeros(gpc + 1, dtype=np.int64)
    np.cumsum(tiles.ravel(), out=tile_off[1:])
    TT = int(tile_off[-1])                    # total tiles per core

    order = np.argsort(g, kind="stable")
    g_s = g[order]
    csum = np.zeros(n_cores * gpc + 1, dtype=np.int64)
    np.cumsum(counts, out=csum[1:])
    rank = np.arange(len(rows), dtype=np.int64) - csum[g_s]
    slot = tile_off[g_s % gpc] * blk + rank
    core_s = g_s // gpc

    idx_slots = np.zeros((n_cores, TT * blk), np.int16)
    dest_slots = np.full((n_cores, TT * blk), 200.0, np.float32)
    val_slots = np.zeros((n_cores, TT * blk), np.float32)
    idx_slots[core_s, slot] = (cols[order] - chunk[order] * cfg.chunk_size).astype(
        np.int16
    )
    dest_slots[core_s, slot] = (rows[order] % blk).astype(np.float32)
    val_slots[core_s, slot] = vals[order]

    # wrapped int16 index buffer, per (sb, chunk) segment
    idx_bufs, dv_bufs = [], []
    # per-(sb,chunk) tile base within the global tile order
    seg_tiles = tiles.sum(axis=2)             # [NSB, NC]
    for c in range(n_cores):
        cols_out = []
        t0 = 0
        for s in range(NSB):
            for ch in range(NC):
                T = int(seg_tiles[s, ch])
                if T == 0:
                    continue
                seg = idx_slots[c, t0 * blk:(t0 + T) * blk]
                w = seg.reshape(-1, 16).T     # idx i -> [i%16, i//16]
                cols_out.append(np.tile(w, (8, 1)))
                t0 += T
        idx_bufs.append(
            np.concatenate(cols_out, axis=1) if cols_out
            else np.zeros((128, 0), np.int16)
        )
        d = dest_slots[c].reshape(TT, blk).T  # [128, TT]
        v = val_slots[c].reshape(TT, blk).T
        dv = np.empty((128, 2 * TT), np.float32)
        dv[:, 0::2] = d
        dv[:, 1::2] = v
        dv_bufs.append(dv)

    rowsum = np.bincount(
        rows, weights=vals.astype(np.float64), minlength=n_cores * D
    ).reshape(n_cores, D).astype(np.float16)

    return tiles, TT, idx_bufs, dv_bufs, rowsum


def _build_nc(cfg: Cfg, tiles, TT, no_gather=False):
    """Build the SPMD Bass program (identical for all cores)."""
    NSB, NC, SBB = cfg.n_sb, cfg.n_chunks, cfg.sb_blocks
    F, blk, D, B = cfg.feat, cfg.blk, cfg.dests_per_core, cfg.blocks_per_core
    FH = F // 128                             # feature halves (2 for F=256)
    seg_tiles = tiles.sum(axis=2)             # [NSB, NC]
    sb_tiles = seg_tiles.sum(axis=1)          # [NSB]

    nc = bacc.Bacc("TRN2", target_bir_lowering=False, debug=False,
                   num_devices=cfg.n_cores)
    xf16 = nc.dram_tensor("xf16", [cfg.n_nodes, F], F16, kind="ExternalInput")
    idx_d = nc.dram_tensor("idx", [128, 8 * TT], I16, kind="ExternalInput")
    dv_d = nc.dram_tensor("dv", [128, 2 * TT], F32, kind="ExternalInput")
    rs_d = nc.dram_tensor("rowsum", [1, D], F16, kind="ExternalInput")
    w_d = nc.dram_tensor("w", [128, FH, F], F16, kind="ExternalInput")
    b_d = nc.dram_tensor("bias", [1, F], F16, kind="ExternalInput")
    iota_d = nc.dram_tensor("iota", [128, 128], F16, kind="ExternalInput")
    out_d = nc.dram_tensor("out", [D, F], F32, kind="ExternalOutput")

    with tile.TileContext(nc) as tc, ExitStack() as ctx:
        const = ctx.enter_context(tc.tile_pool(name="const", bufs=1))
        gath_pool = ctx.enter_context(tc.tile_pool(name="gath", bufs=cfg.gath_bufs))
        idx_pool = ctx.enter_context(tc.tile_pool(name="idxp", bufs=cfg.gath_bufs))
        dv_pool = ctx.enter_context(tc.tile_pool(name="dvp", bufs=3))
        p_pool = ctx.enter_context(tc.tile_pool(name="pp", bufs=8))
        aggsb_pool = ctx.enter_context(tc.tile_pool(name="aggsb", bufs=4))
        outsb_pool = ctx.enter_context(tc.tile_pool(name="outsb", bufs=4))
        aggps_pool = ctx.enter_context(
            tc.tile_pool(name="aggps", bufs=SBB + 1, space="PSUM"))
        outps_pool = ctx.enter_context(
            tc.tile_pool(name="outps", bufs=2, space="PSUM"))

        iota_sb = const.tile([128, 128], F16)
        nc.sync.dma_start(iota_sb[:], iota_d[:])
        w_sb = const.tile([128, FH, F], F16)
        nc.sync.dma_start(w_sb[:], w_d[:])
        b_sb = const.tile([1, F], F16)
        nc.sync.dma_start(b_sb[:], b_d[:])
        rs_sb = const.tile([1, D], F16)
        nc.sync.dma_start(rs_sb[:], rs_d[:])

        seg_tile_base = 0                     # global tile index of segment start
        sb_tile_base = 0
        for s in range(NSB):
            nblk = min(SBB, B - s * SBB)
            # one gather per source chunk covering the whole superblock
            gtiles = []
            for ch in range(NC):
                T = int(seg_tiles[s, ch])
                if T == 0:
                    gtiles.append(None)
                    seg_bases = None
                    continue
                n_idx = T * blk
                it = idx_pool.tile([128, 8 * T], I16, tag="idxp")
                nc.sync.dma_start(
                    it[:], idx_d[:, 8 * seg_tile_base: 8 * (seg_tile_base + T)])
                gt = gath_pool.tile([128, T, F], F16, tag="gath")
                lo = ch * cfg.chunk_size
                hi = min(cfg.n_nodes, (ch + 1) * cfg.chunk_size)
                if no_gather:
                    nc.vector.memset(gt[:], 0.25)
                else:
                    # HW limit: <=1024 indices per dma_gather
                    for k in range(0, T, 8):
                        kt = min(8, T - k)
                        nc.gpsimd.dma_gather(
                            gt[:, k:k + kt, :], xf16[lo:hi, :],
                            it[:, 8 * k: 8 * (k + kt)],
                            kt * blk, kt * blk, F)
                gtiles.append(gt)
                seg_tile_base += T

            Tsb = int(sb_tiles[s])
            dvt = dv_pool.tile([128, 2 * Tsb], F32, tag="dvp")
            nc.sync.dma_start(
                dvt[:], dv_d[:, 2 * sb_tile_base: 2 * (sb_tile_base + Tsb)])

            aggps = []
            for _b in range(nblk):
                # full 2KB PSUM bank per block: halves must not share a
                # zero region with another block's accumulation group
                agg_t = aggps_pool.tile([128, 512], F32, tag="aggps")
                aggps.append(agg_t)
            # last (chunk, tile) per block, for stop flags
            n_left = [int(tiles[s, :, b].sum()) for b in range(nblk)]
            started = [False] * nblk
            dv_col = 0                        # tile index within superblock
            for ch in range(NC):
                gcol = 0
                for b in range(nblk):
                    for _t in range(int(tiles[s, ch, b])):
                        P = p_pool.tile([128, 128], F16, tag="pp")
                        nc.any.tensor_scalar(
                            P[:], iota_sb[:],
                            dvt[:, 2 * dv_col: 2 * dv_col + 1],
                            dvt[:, 2 * dv_col + 1: 2 * dv_col + 2],
                            mybir.AluOpType.is_equal, mybir.AluOpType.mult)
                        n_left[b] -= 1
                        for h in range(FH):
                            nc.tensor.matmul(
                                aggps[b][:, h * 128:(h + 1) * 128],
                                gtiles[ch][:, gcol, h * 128:(h + 1) * 128],
                                P[:],
                                start=(not started[b]) and h == 0,
                                stop=(n_left[b] == 0 and h == FH - 1))
                        started[b] = True
                        dv_col += 1
                        gcol += 1
            sb_tile_base += Tsb

            for b in range(nblk):
                blk_id = s * SBB + b
                if not started[b]:
                    continue                  # no edges anywhere: leave zeros
                asb = aggsb_pool.tile([128, FH * 128], F16, tag="aggsb")
                nc.vector.tensor_copy(asb[:], aggps[b][:, 0:FH * 128])
                ops = outps_pool.tile([128, 512], F32, tag="outps")
                for h in range(FH):
                    nc.tensor.matmul(ops[:, 0:F], asb[:, h * 128:(h + 1) * 128],
                                     w_sb[:, h, :], start=(h == 0), stop=False)
                nc.tensor.matmul(
                    ops[:, 0:F], rs_sb[:, blk_id * 128:(blk_id + 1) * 128], b_sb[:],
                    start=False, stop=True)
                osb = outsb_pool.tile([128, F], F32, tag="outsb")
                nc.vector.tensor_copy(osb[:], ops[:, 0:F])
                nc.sync.dma_start(out_d[blk_id * 128:(blk_id + 1) * 128, :], osb[:])

    nc.compile()
    return nc


def _run(x, rows, cols, vals, weight, b, cfg: Cfg, trace=False, trace_kwargs=None):
    F = cfg.feat
    tiles, TT, idx_bufs, dv_bufs, rowsum = _prepare(rows, cols, vals, cfg)
    nc = _build_nc(cfg, tiles, TT)

    x_f16 = np.ascontiguousarray(np.asarray(x, dtype=np.float16))
    w16 = np.asarray(weight, dtype=np.float16)       # [F, F]
    FH = F // 128
    w_packed = np.ascontiguousarray(
        w16.reshape(FH, 128, F).transpose(1, 0, 2))  # [128, FH, F]
    b16 = np.asarray(b, dtype=np.float16).reshape(1, F)
    iota = np.tile(np.arange(128, dtype=np.float16)[None, :], (128, 1))

    in_maps = []
    for c in range(cfg.n_cores):
        in_maps.append({
            "xf16": x_f16,
            "idx": np.ascontiguousarray(idx_bufs[c]),
            "dv": np.ascontiguousarray(dv_bufs[c]),
            "rowsum": rowsum[c:c + 1],
            "w": w_packed,
            "bias": b16,
            "iota": iota,
        })
    kw = {}
    if trace:
        kw = dict(trace=True, trace_kwargs=trace_kwargs or {})
    res = run_bass_kernel_spmd(nc, in_maps, list(range(cfg.n_cores)), **kw)
    out = np.concatenate([res.results[c]["out"] for c in range(cfg.n_cores)],
                         axis=0)[:cfg.n_nodes]
    return out, res


def kernel(x, G_rows, G_cols, G_vals, weight, bias):
    cfg = Cfg()
    assert x.shape == (cfg.n_nodes, cfg.feat)
    out, _ = _run(x, G_rows, G_cols, G_vals, weight, bias, cfg)
    recon = np.asarray(0, dtype=np.int32)
    return out, recon
